# revision 19
# baseline (speedup 1.0000x reference)
"""IrregularRNN (exact LTC cell) Trainium2 Bass kernel.

Strategy: tensor-parallel split of the 2U=2048 pre-activation columns
across 8 cores. Core k computes pre columns {f: [k*128,(k+1)*128),
a: [U+k*128, U+(k+1)*128)} for the FULL batch B=128, updates h columns
[k*128,(k+1)*128), transposes its h'-slice on the PE, and AllGathers
the transposed slices so every core has the full h^T for the next
step's h @ Wh matmul.

This environment executes through an instruction-level simulator whose
wall cost is dominated by a fixed ~30-80us per instruction, so the
kernel minimizes per-step instruction count:
  - one tanh over the whole [B, 2*SL] pre tile (sigmoid(x) =
    0.5 + 0.5*tanh(x/2); the 0.5 input scale is folded into the
    f-columns of Wx/Wh host-side)
  - no bias matmul when b == 0 (the reference uses b = zeros); a
    one-instruction DVE add fallback otherwise
  - ACT-engine copy PSUM->SBUF after the transpose (same act table set
    as Tanh/Exp, no table switch)
  - ONE strided DMA to load the gathered h^T as [128, 8, B] instead of
    8 per-chunk DMAs
  - ys stores batched: h' accumulates in an SBUF ring [B, 8, SL] and is
    flushed to DRAM once per 8 steps
"""

import sys

sys.path.insert(0, "/opt/trn_rl_repo")

import numpy as np

B, T, D, U = 128, 256, 256, 1024
NC = 8
SL = U // NC          # h columns per core (128)
PW = 2 * SL           # pre-activation columns per core (256)
DK = D // 128         # K-chunks for x part (2)
UK = U // 128         # K-chunks for h part (8)
YS_BATCH = 8          # steps of h' per ys DMA flush

_CACHE: dict = {}


def _build(n_steps: int, with_bias: bool = False, repeat: int = 1,
           use_collective: bool = True, xtiny: bool = False):
    """Build + bacc-compile the SPMD Bass module for n_steps timesteps.

    repeat>1 is a timing-only mode: the T-loop body runs repeat times over
    the same inputs/outputs (numerically wrong; isolates on-device exec
    time from harness data-shipping via wall(2x) - wall(1x))."""
    import concourse.bacc as bacc
    import concourse.tile as tile
    from concourse import mybir

    AF = mybir.ActivationFunctionType
    f32 = mybir.dt.float32

    nc = bacc.Bacc(
        "TRN2",
        target_bir_lowering=False,
        debug=False,
        enable_asserts=False,
        num_devices=NC,
    )

    # --- kernel I/O ---------------------------------------------------
    n_xt = YS_BATCH if xtiny else n_steps
    xT = nc.dram_tensor("xT", [n_xt, DK, 128, B], f32, kind="ExternalInput")
    wx_sl = nc.dram_tensor("wx_sl", [DK, 128, PW], f32, kind="ExternalInput")
    wh_sl = nc.dram_tensor("wh_sl", [UK, 128, PW], f32, kind="ExternalInput")
    b_sl = nc.dram_tensor("b_sl", [B, PW], f32, kind="ExternalInput")
    tau_b = nc.dram_tensor("tau_b", [B, SL], f32, kind="ExternalInput")
    ndt = nc.dram_tensor("ndt", [B, n_xt], f32, kind="ExternalInput")
    h0T = nc.dram_tensor("h0T", [UK, 128, B], f32, kind="ExternalInput")
    h0_sl = nc.dram_tensor("h0_sl", [B, SL], f32, kind="ExternalInput")
    ident = nc.dram_tensor("ident", [128, 128], f32, kind="ExternalInput")
    ys_sl = nc.dram_tensor("ys_sl", [n_steps, B, SL], f32, kind="ExternalOutput")

    RG = [list(range(NC))]

    with tile.TileContext(nc) as tc:
        with (
            tc.tile_pool(name="const", bufs=1) as cpool,
            tc.tile_pool(name="xin", bufs=6) as xpool,
            tc.tile_pool(name="hT", bufs=2) as hTpool,
            tc.tile_pool(name="act", bufs=3) as apool,
            tc.tile_pool(name="hbig", bufs=2) as hbpool,
            tc.tile_pool(name="pre", bufs=2, space="PSUM") as prepool,
            tc.tile_pool(name="trp", bufs=2, space="PSUM") as trpool,
            tc.tile_pool(name="agio", bufs=2, space="DRAM") as dpool,
            tc.tile_pool(name="pxd", bufs=1, space="DRAM") as pxpool,
        ):
            # --- constants, loaded once -------------------------------
            wx_sb = cpool.tile([128, DK, PW], f32, name="wx_sb")
            nc.sync.dma_start(out=wx_sb[:], in_=wx_sl.ap().rearrange("c p n -> p c n"))
            wh_sb = cpool.tile([128, UK, PW], f32, name="wh_sb")
            nc.sync.dma_start(out=wh_sb[:], in_=wh_sl.ap().rearrange("c p n -> p c n"))
            tau_sb = cpool.tile([B, SL], f32, name="tau_sb")
            nc.sync.dma_start(out=tau_sb[:], in_=tau_b[:])
            ndt_sb = cpool.tile([B, n_xt], f32, name="ndt_sb")
            nc.sync.dma_start(out=ndt_sb[:], in_=ndt[:])
            ident_sb = cpool.tile([128, 128], f32, name="ident_sb")
            nc.sync.dma_start(out=ident_sb[:], in_=ident[:])
            if with_bias:
                b_sb = cpool.tile([B, PW], f32, name="b_sb")
                nc.sync.dma_start(out=b_sb[:], in_=b_sl[:])

            # initial state
            h_prev = cpool.tile([B, SL], f32, name="h0_sb")
            nc.sync.dma_start(out=h_prev[:], in_=h0_sl[:])
            hT_cur = cpool.tile([128, UK, B], f32, name="hT0_sb")
            nc.sync.dma_start(out=hT_cur[:], in_=h0T.ap().rearrange("c p b -> p c b"))

            # --- precompute preX[t] = x_t @ Wx (+ b) for all t --------
            # h-independent, so it runs before the recurrence; the serial
            # loop then only pays one DVE add per step for the x part.
            preX_dram = pxpool.tile([n_xt, B, PW], f32, name="preX")
            XPB = 8  # t's per precompute tile group
            for t0p in range(0, n_xt, XPB):
                xt8 = xpool.tile([128, XPB, DK, B], f32, name="xt8")
                nc.sync.dma_start(
                    out=xt8[:],
                    in_=xT[t0p : t0p + XPB].rearrange("s c p b -> p s c b"),
                )
                for s in range(0, XPB, 2):
                    # two steps per PSUM tile -> one copy + one store per 2t
                    px = prepool.tile([B, 2, PW], f32, name="px")
                    for u in range(2):
                        for c in range(DK):
                            nc.tensor.matmul(
                                px[:, u, :], xt8[:, s + u, c, :], wx_sb[:, c, :],
                                start=(c == 0), stop=(c == DK - 1),
                            )
                    pxs = apool.tile([B, 2, PW], f32, name="pxs")
                    if with_bias:
                        for u in range(2):
                            nc.vector.tensor_add(
                                pxs[:, u, :], px[:, u, :], b_sb[:]
                            )
                    else:
                        nc.scalar.activation(pxs[:], px[:], AF.Copy)
                    nc.sync.dma_start(
                        out=preX_dram[t0p + s : t0p + s + 2].rearrange(
                            "s b n -> b s n"
                        ),
                        in_=pxs[:],
                    )

            hbig = None
            px8 = None
            # --- the recurrence ---------------------------------------
            for tv in range(n_steps * repeat):
                t = tv % n_steps
                tx = t % YS_BATCH if xtiny else t
                s8 = t % YS_BATCH
                if s8 == 0:
                    # batched load: 8 steps of precomputed x-part
                    t0x = 0 if xtiny else t
                    px8 = xpool.tile([B, YS_BATCH, PW], f32, name="px8")
                    nc.sync.dma_start(
                        out=px8[:],
                        in_=preX_dram[t0x : t0x + YS_BATCH].rearrange(
                            "s b n -> b s n"
                        ),
                    )

                pre = prepool.tile([B, PW], f32, name="pre")
                for j in range(UK):
                    nc.tensor.matmul(
                        pre[:],
                        hT_cur[:, j, :],
                        wh_sb[:, j, :],
                        start=(j == 0),
                        stop=(j == UK - 1),
                    )

                # add the precomputed x part
                ps = apool.tile([B, PW], f32, name="ps")
                nc.vector.tensor_add(ps[:], pre[:], px8[:, s8, :])

                # one tanh over the whole pre tile:
                #   cols [0,SL)  = tanh(0.5*pre_f)  (0.5 folded into weights)
                #   cols [SL,PW) = tanh(pre_a) = a
                ta = apool.tile([B, PW], f32, name="ta")
                nc.scalar.activation(ta[:], ps[:], AF.Tanh)
                a = ta[:, SL:PW]
                # g = tau + sigmoid(pre_f) = (tau + 0.5) + 0.5*tanh(pre_f/2)
                g = apool.tile([B, SL], f32, name="g")
                nc.vector.scalar_tensor_tensor(
                    g[:], ta[:, 0:SL], 0.5, tau_sb[:],
                    mybir.AluOpType.mult, mybir.AluOpType.add,
                )
                dcy = apool.tile([B, SL], f32, name="dcy")
                nc.scalar.activation(
                    dcy[:], g[:], AF.Exp, scale=ndt_sb[:, tx : tx + 1]
                )
                hma = apool.tile([B, SL], f32, name="hma")
                nc.vector.tensor_sub(hma[:], h_prev[:], a)
                hd = apool.tile([B, SL], f32, name="hd")
                nc.vector.tensor_mul(hd[:], hma[:], dcy[:])
                if s8 == 0:
                    hbig = hbpool.tile([B, YS_BATCH, SL], f32, name="hbig")
                h_new = hbig[:, s8, :]
                nc.vector.tensor_add(h_new, hd[:], a)

                if s8 == YS_BATCH - 1:
                    # flush YS_BATCH steps of h' in one strided DMA
                    t0 = t - (YS_BATCH - 1)
                    nc.sync.dma_start(
                        out=ys_sl[t0 : t + 1].rearrange("s b u -> b s u"),
                        in_=hbig[:],
                    )

                if tv == n_steps * repeat - 1:
                    h_prev = h_new
                    break

                # h'^T slice for the next step's matmul
                trp = trpool.tile([128, B], f32, name="trp")
                nc.tensor.transpose(trp[:], h_new, ident_sb[:])
                trs = apool.tile([128, B], f32, name="trs")
                nc.scalar.activation(trs[:], trp[:], AF.Copy)
                ag_in = dpool.tile([128, B], f32, name="ag_in")
                nc.sync.dma_start(out=ag_in[:], in_=trs[:])
                hT_next = hTpool.tile([128, UK, B], f32, name="hTg")
                if use_collective:
                    ag_out = dpool.tile(
                        [UK * 128, B], f32, name="ag_out", addr_space="Shared"
                    )
                    nc.gpsimd.collective_compute(
                        "AllGather",
                        mybir.AluOpType.bypass,
                        replica_groups=RG,
                        ins=[ag_in[:].opt()],
                        outs=[ag_out[:].opt()],
                    )
                    nc.sync.dma_start(
                        out=hT_next[:],
                        in_=ag_out[:].rearrange("(c p) b -> p c b", p=128),
                    )
                else:
                    # timing-only bisect variant: local slice in place of
                    # the gathered one (numerically wrong on purpose)
                    nc.sync.dma_start(
                        out=hT_next[:],
                        in_=ag_in[:].rearrange("p (c b) -> p c b", c=1).broadcast_to(
                            [128, UK, B]
                        ),
                    )
                hT_cur = hT_next
                h_prev = h_new

    nc.compile()
    return nc


def _prep_inputs(features, time_steps, Wx, Wh, b, w_tau, h0, n_steps):
    """Host-side sharding + layout transforms -> per-core in_maps."""
    f32 = np.float32
    features = np.asarray(features, dtype=f32)
    time_steps = np.asarray(time_steps, dtype=f32)
    Wx = np.asarray(Wx, dtype=f32)
    Wh = np.asarray(Wh, dtype=f32)
    b = np.asarray(b, dtype=f32)
    w_tau = np.asarray(w_tau, dtype=f32)
    h0 = np.asarray(h0, dtype=f32)

    # softplus(w_tau), fp32
    tau = np.log1p(np.exp(w_tau)).astype(f32)

    # fold the sigmoid half-angle scale into the f columns
    Wxs = Wx.copy()
    Wxs[:, :U] *= 0.5
    Whs = Wh.copy()
    Whs[:, :U] *= 0.5
    bs = b.copy()
    bs[:U] *= 0.5
    with_bias = bool(np.any(b != 0.0))

    xT = np.ascontiguousarray(features.transpose(1, 2, 0)).reshape(n_steps, DK, 128, B)
    ndt = np.ascontiguousarray(-time_steps)                      # [B, T]
    h0T = np.ascontiguousarray(h0.T).reshape(UK, 128, B)
    ident = np.eye(128, dtype=f32)

    in_maps = []
    for k in range(NC):
        cols = np.concatenate(
            [np.arange(k * SL, (k + 1) * SL), U + np.arange(k * SL, (k + 1) * SL)]
        )
        in_maps.append(
            {
                "xT": xT,
                "wx_sl": np.ascontiguousarray(Wxs[:, cols]).reshape(DK, 128, PW),
                "wh_sl": np.ascontiguousarray(Whs[:, cols]).reshape(UK, 128, PW),
                "b_sl": np.ascontiguousarray(
                    np.broadcast_to(bs[cols], (B, PW))
                ),
                "tau_b": np.ascontiguousarray(
                    np.broadcast_to(tau[k * SL : (k + 1) * SL] + 0.5, (B, SL))
                ),
                "ndt": ndt,
                "h0T": h0T,
                "h0_sl": np.ascontiguousarray(h0[:, k * SL : (k + 1) * SL]),
                "ident": ident,
            }
        )
    return in_maps, with_bias


def _assemble(results):
    """[T, B, SL] slices per core -> [B, T, U] full output."""
    ys = np.concatenate([r["ys_sl"] for r in results], axis=2)  # [T, B, U]
    return np.ascontiguousarray(ys.transpose(1, 0, 2))


def kernel(features, time_steps, Wx, Wh, b, w_tau, h0, _trace=False, _repeat=1):
    from concourse import bass_utils

    n_steps = features.shape[1]
    in_maps, with_bias = _prep_inputs(
        features, time_steps, Wx, Wh, b, w_tau, h0, n_steps
    )
    key = (n_steps, with_bias, _repeat)
    if key not in _CACHE:
        _CACHE[key] = _build(n_steps, with_bias=with_bias, repeat=_repeat)
    nc = _CACHE[key]

    try:
        res = bass_utils.run_bass_kernel_spmd(
            nc, in_maps, core_ids=list(range(NC)), trace=_trace
        )
    except ModuleNotFoundError:
        # no NTFF profiling hook in this container — run untraced
        res = bass_utils.run_bass_kernel_spmd(
            nc, in_maps, core_ids=list(range(NC)), trace=False
        )
    out = _assemble(res.results)
    if _trace:
        return out, res
    return out


if __name__ == "__main__":
    # smoke test with random data
    rng = np.random.default_rng(0)
    feats = rng.standard_normal((B, T, D), dtype=np.float32)
    ts = rng.random((B, T), dtype=np.float32)
    Wx = rng.standard_normal((D, 2 * U), dtype=np.float32) / np.sqrt(D)
    Wh = rng.standard_normal((U, 2 * U), dtype=np.float32) / np.sqrt(U)
    b = np.zeros((2 * U,), dtype=np.float32)
    w_tau = rng.random((U,), dtype=np.float32)
    h0 = np.zeros((B, U), dtype=np.float32)
    out = kernel(feats, ts, Wx, Wh, b, w_tau, h0)
    print("output", out.shape, out.dtype)
